# revision 31
# baseline (speedup 1.0000x reference)
"""Bass/Trainium2 kernel for nn_BaseAttention (B=2, N=2048, D=1024, H=16 causal).

Sharding: batch x head-group. Core c handles batch c//4 and heads
[4*(c%4), 4*(c%4)+4) (feature slice of 256 = 2 head-pairs). Each core
computes q/k/v projections for its slice from its batch's (pre-transposed)
x, runs causal attention for its 4 heads, applies its slice of the output
projection (row-parallel Wo), and writes a bf16 [2048, 1024] partial that
the host sums per batch.

Device dataflow (per core, matmuls bf16 -> fp32 PSUM):
  xt (SBUF)  --matmul-->  qT, kT   [2 pair-chunks x 128 feats, 2048 toks]
             --matmul-->  V directly in [tok, feat] layout (xt tile is the
                          stationary operand), stored per 128-tok k-tile as
                          4 x [V_h (64) | ones] slots for the EV matmuls
  For each (q-chunk, pair, k-tile):
    S^T[k, q] for BOTH heads of the pair in one PSUM tile [128, 1024]:
      head0 uses PE row group 0-1 (contraction partitions 0:64),
      head1 row group 2-3 (partitions 64:128) -- the two K=64 matmuls run
      concurrently via tile_position auto-derivation, costing ~1x N cycles.
    E^T = exp(S^T) on ACT (no max subtraction: |logits| < ~4 for this data)
    [O_h; den_h] = [V_h | 1].T @ E_h^T accumulated over k-tiles (den rides
      as the 65th matmul row, costing no extra PE time)
  O_norm = O * (1/den);  1/den = exp(-ln(den)) on ACT (custom-DVE
    reciprocal is broken on HW), broadcast across the 64 feat partitions
    via a DRAM-roundtrip DMA with a 0-stride AP (PE ones-matmul broadcast
    for the final pair, to shorten the kernel tail).
  out_partial[tok, :] = O_norm^T.T @ WoT_slice  (bf16)

Scheduling: the attention inner loop is ACT(exp)-bound (~1us/tile) while
the PE needs only ~640ns/tile, so projection chunk nt+1 and the Wo of
q-chunk qc-1 are emitted as generator "streams" dripped 1-2 matmuls at a
time between k-tile iterations -- fine enough that the QK->exp cadence
never starves the ACT engine. V-projections have the latest deadline
(EV of k-tile j) and are dripped last-deadline-first is not needed;
q (needed at next chunk's start), then k, then V, then Wo.
"""

import contextlib
import itertools
import numpy as np
import ml_dtypes

B, N, D = 2, 2048, 1024
H, DH = 16, 64
NCORES = 8
CPB = 4                  # cores per batch
HPC = H // CPB           # heads per core
F = HPC * DH             # feature slice per core (256)
T = N                    # tokens per core (one batch)
NPAIR = HPC // 2         # head pairs per core
SCALE = DH ** -0.5
P = 128
KC = D // P              # k-chunks over the model dim
NT = T // 512            # 512-token chunks
QC = N // 512            # q chunks per batch
JT = N // P              # 128-token k tiles per batch

BF16 = ml_dtypes.bfloat16

_BUILT = {}


def _build_program(debug=False, loop=0):
    import concourse.bass as bass
    import concourse.tile as tile
    from concourse import mybir
    from concourse.bacc import Bacc

    f32 = mybir.dt.float32
    bf16 = mybir.dt.bfloat16
    EXP = mybir.ActivationFunctionType.Exp

    class BaccOneActTable(Bacc):
        """Force every activation onto the natural_log_exp_and_others table
        set (it contains Exp, Ln, Copy and Identity) so the ACT engine loads
        its function table exactly once instead of thrashing between the
        exp and ln sets (~2.7us per reload)."""

        def insert_act_table_loads(self):
            import bass_rust as _bass_rust
            from concourse.hw_specs import get_activation_tables

            has_activation = any(
                isinstance(i, mybir.InstActivation)
                for blk in self.main_func.blocks
                for i in blk.instructions
            )
            if not has_activation:
                return
            keep = "natural_log_exp_and_others"
            tables = [
                (nm, (fns if nm == keep else set()))
                for nm, fns in get_activation_tables(self.m.arch).items()
            ]
            _bass_rust.insert_act_table_loads(self, tables)

    nc = BaccOneActTable()
    xt = nc.declare_dram_parameter("xt", [D, T], bf16, isOutput=False)
    wq = nc.declare_dram_parameter("wq", [D, F], bf16, isOutput=False)
    wk = nc.declare_dram_parameter("wk", [D, F], bf16, isOutput=False)
    wv = nc.declare_dram_parameter("wv", [D, F], bf16, isOutput=False)
    wo = nc.declare_dram_parameter("wo", [F, D], bf16, isOutput=False)
    out = nc.declare_dram_parameter("out", [T, D], bf16, isOutput=True)

    with tile.TileContext(nc) as tc:
        with contextlib.ExitStack() as ctx:
            persist = ctx.enter_context(tc.tile_pool(name="persist", bufs=1))
            work = ctx.enter_context(tc.tile_pool(name="work", bufs=1))
            dpool = ctx.enter_context(
                tc.tile_pool(name="dscratch", bufs=1, space="DRAM"))

            # ---- persistent SBUF tensors ----
            xt_sb = persist.tile([P, KC, T], bf16)          # x^T, chunked over D
            wq_sb = persist.tile([P, KC, F], bf16)
            wk_sb = persist.tile([P, KC, F], bf16)
            wv_sb = persist.tile([P, KC, F], bf16)
            wo_sb = persist.tile([P, NPAIR, D], bf16)
            qt_sb = persist.tile([P, NPAIR, T], bf16)       # Q^T (scaled)
            kt_sb = persist.tile([P, NPAIR, T], bf16)       # K^T
            # V per 128-tok k-tile: 4 head slots of [V_h (64 cols) | ones]
            va_sb = persist.tile([P, JT, HPC, DH + 1], bf16)
            ot_sb = persist.tile([P, NPAIR, T], bf16)       # normalized O^T
            et_all = persist.tile([P, 5, 1024], bf16)       # E^T rotation bufs
            # causal keep-mask for the diagonal 128x128 block: 1 iff c >= p
            trimask = persist.tile([P, P], bf16)
            ones64 = persist.tile([DH + 1, DH], f32)        # PE bcast lhsT

            # ---- constants ----
            nc.gpsimd.memset(trimask, 1.0)
            nc.gpsimd.affine_select(
                out=trimask, in_=trimask,
                compare_op=mybir.AluOpType.is_ge,
                fill=0.0, base=0, pattern=[[1, P]], channel_multiplier=-1,
            )
            nc.gpsimd.memset(va_sb[:, :, :, DH], 1.0)
            nc.gpsimd.memset(et_all, 0.0)
            nc.gpsimd.memset(ones64, 1.0)

            psum = tc.alloc_tile_pool(name="psum", bufs=1, space="PSUM")

            def body():
                # ---- HAM pre-warm: dense dummy matmuls during the input-DMA
                # window so the PE clock-gate is at 8/8 (2.4 GHz) by the time
                # the first projection matmul issues (saves the ~2us cold
                # penalty; outputs are garbage and never read) ----
                warm = psum.tile([P, P], f32, tag="wop", bufs=1, name="warm")
                for w in range(20):
                    nc.tensor.matmul(warm, trimask, trimask,
                                     start=True, stop=True)

                # ---- load inputs; first projection chunk's needs first ----
                xt_r = xt.rearrange("(a p) t -> p a t", p=P)
                nc.sync.dma_start(out=xt_sb[:, :, 0:512], in_=xt_r[:, :, 0:512])
                nc.sync.dma_start(out=wq_sb, in_=wq.rearrange("(a p) f -> p a f", p=P))
                nc.sync.dma_start(out=wk_sb, in_=wk.rearrange("(a p) f -> p a f", p=P))
                nc.sync.dma_start(out=wv_sb, in_=wv.rearrange("(a p) f -> p a f", p=P))
                for nt in range(1, NT):
                    nc.sync.dma_start(
                        out=xt_sb[:, :, nt * 512:(nt + 1) * 512],
                        in_=xt_r[:, :, nt * 512:(nt + 1) * 512])
                nc.sync.dma_start(out=wo_sb, in_=wo.rearrange("(a p) f -> p a f", p=P))

                # ---- projection / Wo generator streams (for dripping) ----
                def qk_stream(nt, p2, w_sb, dst, ptag, nm):
                    c0 = nt * 512
                    fsl = slice(p2 * P, (p2 + 1) * P)
                    pp = psum.tile([P, 512], f32, tag=ptag, bufs=1,
                                   name=f"pj{nt}{p2}{nm}")
                    for kc in range(KC):
                        nc.tensor.matmul(
                            pp, w_sb[:, kc, fsl], xt_sb[:, kc, c0:c0 + 512],
                            start=(kc == 0), stop=(kc == KC - 1))
                        yield
                    nc.vector.tensor_copy(dst[:, p2, c0:c0 + 512], pp)
                    yield

                def v_stream(tt, ptag):
                    t0 = tt * P
                    pv = psum.tile([P, HPC, DH], f32, tag=ptag, bufs=1,
                                   name=f"pv{tt}")
                    for kc in range(KC):
                        nc.tensor.matmul(
                            pv, xt_sb[:, kc, t0:t0 + P], wv_sb[:, kc, :],
                            start=(kc == 0), stop=(kc == KC - 1))
                        if kc % 2 == 1:
                            yield
                    nc.vector.tensor_copy(va_sb[:, tt, :, 0:DH], pv)
                    yield

                def wo_stream(t0, ptag, pbufs=1):
                    # half-token-tile units so the PSUM tile is one bank
                    for nn in range(2):
                        nsl = slice(nn * 512, (nn + 1) * 512)
                        wop = psum.tile([P, 512], f32, tag=ptag, bufs=pbufs,
                                        name=f"wop{t0}_{nn}")
                        nc.tensor.matmul(wop, ot_sb[:, 0, t0:t0 + P],
                                         wo_sb[:, 0, nsl], start=True,
                                         stop=False)
                        yield
                        nc.tensor.matmul(wop, ot_sb[:, 1, t0:t0 + P],
                                         wo_sb[:, 1, nsl], start=False,
                                         stop=True)
                        yield
                        wos = work.tile([P, 512], bf16, tag="wos", bufs=3,
                                        name=f"wos{t0}_{nn}")
                        nc.vector.tensor_copy(wos, wop)
                        nc.sync.dma_start(out=out[t0:t0 + P, nsl], in_=wos)
                        yield

                def stream_for(qc):
                    """Work dripped during attn(qc), allocated so the PE-idle
                    slack of the late (ACT-bound, large-jmax) q-chunks is
                    filled while respecting deadlines: q(c) by attn(c) start,
                    k(c)/V(c) by attn(c)'s k-tile 4c, wo(c) any time after
                    attn(c)'s normalization. Earliest-deadline work first
                    within each stream."""
                    gens = []
                    tags = itertools.cycle(["wop", "wop2"])

                    def q_gens(c):
                        return [qk_stream(c, p2, wq_sb, qt_sb, next(tags), "q")
                                for p2 in range(NPAIR)]

                    def k_gens(c):
                        return [qk_stream(c, p2, wk_sb, kt_sb, next(tags), "k")
                                for p2 in range(NPAIR)]

                    def v_gens(c):
                        return [v_stream(c * 4 + j4, next(tags))
                                for j4 in range(4)]

                    def wo_gens(c):
                        return [wo_stream(c * 512 + ti * P, next(tags))
                                for ti in range(4)]

                    if qc == 0:
                        gens = q_gens(1) + k_gens(1) + v_gens(1)
                        steps = 56
                    elif qc == 1:
                        gens = q_gens(2) + k_gens(2) + v_gens(2)
                        steps = 56
                    elif qc == 2:
                        gens = q_gens(3) + k_gens(3) + v_gens(3) + wo_gens(0)
                        steps = 80
                    else:
                        gens = wo_gens(1) + wo_gens(2)
                        steps = 48
                    return itertools.chain(*gens), steps

                # proj(0) upfront, rotating psum tags so groups pipeline
                tags = itertools.cycle(["wop", "wop2", "ov0", "ov1"])
                for p2 in range(NPAIR):
                    for _ in qk_stream(0, p2, wq_sb, qt_sb, next(tags), "q"):
                        pass
                    for _ in qk_stream(0, p2, wk_sb, kt_sb, next(tags), "k"):
                        pass
                for j4 in range(4):
                    for _ in v_stream(j4, next(tags)):
                        pass

                et_idx = 0
                for qc in range(QC):
                    qg = qc * 512
                    jmax = (qc + 1) * 4       # k tiles needed (causal)
                    stream, nsteps = stream_for(qc)
                    ntile = NPAIR * jmax
                    tdone = 0
                    sdone = 0

                    def drip(target):
                        nonlocal sdone
                        while sdone < target:
                            if next(stream, None) is None:
                                sdone = max(sdone, nsteps)
                                break
                            sdone += 1

                    def drain():
                        nonlocal sdone
                        for _ in stream:
                            sdone += 1

                    for pr in range(NPAIR):
                        last = (qc == QC - 1 and pr == NPAIR - 1)
                        ovc = work.tile([DH + 1, 1024], f32, tag="ovc",
                                        bufs=2, name=f"ovc{qc}_{pr}")
                        ovs = []
                        for hh in range(2):
                            ov = psum.tile([P, 512], f32, tag=f"ov{hh}",
                                           bufs=1, name=f"ov{hh}_{qc}{pr}")
                            ovs.append(ov[0:DH + 1, :])  # [O(0:64); den(64)]
                        def emit_ev(j, pfx, et):
                            for hh in range(2):
                                nc.tensor.matmul(
                                    ovs[hh][:, pfx:512],
                                    va_sb[:, j, 2 * pr + hh, :],
                                    et[:, hh * 512 + pfx:(hh + 1) * 512],
                                    start=(j == 0), stop=(j == jmax - 1))

                        # EV emission lags QK by two k-tiles so the PE never
                        # waits on exp(j) -- not even across pair boundaries
                        # where the ACT runs the ~2us reciprocal chain first;
                        # the drip between them fills the remaining slack.
                        pend = []
                        for j in range(jmax):
                            kg = j * P
                            # pfx: fully-masked q-column prefix of this tile
                            pfx = max(0, (j - qc * 4)) * P
                            st = psum.tile([P, 1024], f32, tag="st", bufs=2,
                                           name=f"st{qc}_{pr}_{j}")
                            et = et_all[:, et_idx % 5, :]
                            et_idx += 1
                            for hh in range(2):
                                hsl = slice(hh * DH, (hh + 1) * DH)
                                nc.tensor.matmul(
                                    st[:, hh * 512 + pfx:(hh + 1) * 512],
                                    kt_sb[hsl, pr, kg:kg + P],
                                    qt_sb[hsl, pr, qg + pfx:qg + 512],
                                    start=True, stop=True)
                            if pfx == 0:
                                nc.scalar.activation(et, st, EXP)
                            else:
                                for hh in range(2):
                                    esl = slice(hh * 512 + pfx, (hh + 1) * 512)
                                    nc.scalar.activation(et[:, esl], st[:, esl],
                                                         EXP)
                            if j >= qc * 4:   # diag: causal mask
                                for hh in range(2):
                                    blk = slice(hh * 512 + pfx,
                                                hh * 512 + pfx + P)
                                    nc.vector.tensor_mul(
                                        et[:, blk], et[:, blk], trimask)
                            tdone += 1
                            drip(nsteps * tdone // ntile)
                            pend.append((j, pfx, et))
                            if len(pend) > 2:
                                emit_ev(*pend.pop(0))
                        for ev in pend:
                            emit_ev(*ev)
                        # move [O; den] out of PSUM right away so the ov
                        # slots free for the next pair/q-chunk (final pair:
                        # split across DVE+ACT so the tail copies run in
                        # parallel -- ACT's exp queue is empty by then)
                        nc.vector.tensor_copy(ovc[:, 0:512], ovs[0])
                        if last:
                            nc.scalar.copy(ovc[:, 512:1024], ovs[1])
                        else:
                            nc.vector.tensor_copy(ovc[:, 512:1024], ovs[1])
                        # 1/den = exp(-ln(den)) on ACT. ACT op cost scales
                        # with per-partition free dim, so reshape the 1024
                        # dens (one partition) to [8,128] via a DRAM bounce
                        # before Ln/Exp (~0.3us/op instead of ~1us), then
                        # broadcast over the 64 feat partitions with a
                        # 0-stride AP. The final pair keeps the 1-partition
                        # Ln/Exp + PE ones-matmul (shorter critical tail).
                        if last:
                            lse = work.tile([DH + 1, 1024], f32, tag="lse",
                                            bufs=2, name=f"lse{qc}_{pr}")
                            nc.scalar.activation(
                                lse[DH:DH + 1, :], ovc[DH:DH + 1, :],
                                mybir.ActivationFunctionType.Ln)
                            rden = work.tile([DH + 1, 1024], f32, tag="rden",
                                             bufs=2, name=f"rden{qc}_{pr}")
                            nc.scalar.activation(
                                rden[DH:DH + 1, :], lse[DH:DH + 1, :],
                                mybir.ActivationFunctionType.Exp, scale=-1.0)
                            rbc_ps = psum.tile([DH, 1024], f32, tag="st",
                                               bufs=2, name="rbcps")
                            for nn in range(2):
                                nsl = slice(nn * 512, (nn + 1) * 512)
                                nc.tensor.matmul(
                                    rbc_ps[:, nsl], ones64[DH:DH + 1, :],
                                    rden[DH:DH + 1, nsl],
                                    start=True, stop=True)
                            rbc = rbc_ps
                        else:
                            dsc = dpool.tile([1, 1024], f32, tag="dsc", bufs=2,
                                             name=f"dsc{qc}_{pr}")
                            nc.sync.dma_start(out=dsc, in_=ovc[DH:DH + 1, :])
                            den8 = work.tile([8, P], f32, tag="den8", bufs=2,
                                             name=f"den8_{qc}_{pr}")
                            nc.sync.dma_start(out=den8, in_=bass.AP(
                                tensor=dsc.tensor, offset=dsc.offset,
                                ap=[[P, 8], [1, P]]))
                            lse8 = work.tile([8, P], f32, tag="lse8", bufs=2,
                                             name=f"lse8_{qc}_{pr}")
                            nc.scalar.activation(
                                lse8, den8, mybir.ActivationFunctionType.Ln)
                            rden8 = work.tile([8, P], f32, tag="rden8", bufs=2,
                                              name=f"rden8_{qc}_{pr}")
                            nc.scalar.activation(
                                rden8, lse8, mybir.ActivationFunctionType.Exp,
                                scale=-1.0)
                            dsc2 = dpool.tile([8, P], f32, tag="dsc2", bufs=2,
                                              name=f"dsc2_{qc}_{pr}")
                            nc.sync.dma_start(out=dsc2, in_=rden8)
                            rbc = work.tile([DH, 1024], f32, tag="rbc", bufs=2,
                                            name=f"rbc{qc}_{pr}")
                            bc_ap = bass.AP(
                                tensor=dsc2.tensor, offset=dsc2.offset,
                                ap=[[0, DH], [1, 1024]])
                            nc.sync.dma_start(out=rbc, in_=bc_ap)
                        nc.vector.tensor_mul(
                            ot_sb[0:DH, pr, qg:qg + 512], ovc[0:DH, 0:512],
                            rbc[:, 0:512])
                        otb = work.tile([DH, 512], bf16, tag="otb", bufs=2,
                                        name=f"otb{qc}_{pr}")
                        nc.vector.tensor_mul(otb, ovc[0:DH, 512:1024],
                                             rbc[:, 512:1024])
                        nc.sync.dma_start(out=ot_sb[DH:P, pr, qg:qg + 512],
                                          in_=otb)
                    drain()
                # tail: Wo for the last q-chunk on the (now free) st slots
                for ti in range(4):
                    for _ in wo_stream((QC - 1) * 512 + ti * P, "st", 2):
                        pass

            if loop:
                ET = mybir.EngineType
                with tc.For_i(0, loop, 1,
                              hint_engines=(ET.PE, ET.DVE, ET.Activation,
                                            ET.SP, ET.Pool)):
                    body()
            else:
                body()
            psum.release()

    nc.finalize()
    return nc


def _get_program():
    if "nc" not in _BUILT:
        _BUILT["nc"] = _build_program()
    return _BUILT["nc"]


def _prep_inputs(x, Wq, Wkv, Wo):
    maps = []
    xts = [np.ascontiguousarray(x[b].T).astype(BF16) for b in range(B)]
    for c in range(NCORES):
        b, g = divmod(c, CPB)
        r0 = g * F
        maps.append({
            "xt": xts[b],
            "wq": np.ascontiguousarray(Wq[r0:r0 + F, :].T * SCALE).astype(BF16),
            "wk": np.ascontiguousarray(Wkv[r0:r0 + F, :].T).astype(BF16),
            "wv": np.ascontiguousarray(Wkv[D + r0:D + r0 + F, :].T).astype(BF16),
            "wo": np.ascontiguousarray(Wo[:, r0:r0 + F].T).astype(BF16),
        })
    return maps


def kernel(x, Wq, Wkv, Wo):
    from concourse.bass_utils import run_bass_kernel_spmd

    nc = _get_program()
    in_maps = _prep_inputs(np.asarray(x, np.float32), np.asarray(Wq, np.float32),
                           np.asarray(Wkv, np.float32), np.asarray(Wo, np.float32))
    res = run_bass_kernel_spmd(nc, in_maps, list(range(NCORES)))
    acc = np.zeros((B, N, D), np.float32)
    for c in range(NCORES):
        b = c // CPB
        acc[b] += res.results[c]["out"].astype(np.float32)
    return acc


# revision 32
# speedup vs baseline: 1.0180x; 1.0180x over previous
"""Bass/Trainium2 kernel for nn_BaseAttention (B=2, N=2048, D=1024, H=16 causal).

Sharding: batch x head-group. Core c handles batch c//4 and heads
[4*(c%4), 4*(c%4)+4) (feature slice of 256 = 2 head-pairs). Each core
computes q/k/v projections for its slice from its batch's (pre-transposed)
x, runs causal attention for its 4 heads, applies its slice of the output
projection (row-parallel Wo), and writes a bf16 [2048, 1024] partial that
the host sums per batch.

Device dataflow (per core, matmuls bf16 -> fp32 PSUM):
  xt (SBUF)  --matmul-->  qT, kT   [2 pair-chunks x 128 feats, 2048 toks]
             --matmul-->  V directly in [tok, feat] layout (xt tile is the
                          stationary operand), stored per 128-tok k-tile as
                          4 x [V_h (64) | ones] slots for the EV matmuls
  For each (q-chunk, pair, k-tile):
    S^T[k, q] for BOTH heads of the pair in one PSUM tile [128, 1024]:
      head0 uses PE row group 0-1 (contraction partitions 0:64),
      head1 row group 2-3 (partitions 64:128) -- the two K=64 matmuls run
      concurrently via tile_position auto-derivation, costing ~1x N cycles.
    E^T = exp(S^T) on ACT (no max subtraction: |logits| < ~4 for this data)
    [O_h; den_h] = [V_h | 1].T @ E_h^T accumulated over k-tiles (den rides
      as the 65th matmul row, costing no extra PE time)
  O_norm = O * (1/den);  1/den = exp(-ln(den)) on ACT (custom-DVE
    reciprocal is broken on HW), broadcast across the 64 feat partitions
    via a DRAM-roundtrip DMA with a 0-stride AP (PE ones-matmul broadcast
    for the final pair, to shorten the kernel tail).
  out_partial[tok, :] = O_norm^T.T @ WoT_slice  (bf16)

Scheduling: the attention inner loop is ACT(exp)-bound (~1us/tile) while
the PE needs only ~640ns/tile, so projection chunk nt+1 and the Wo of
q-chunk qc-1 are emitted as generator "streams" dripped 1-2 matmuls at a
time between k-tile iterations -- fine enough that the QK->exp cadence
never starves the ACT engine. V-projections have the latest deadline
(EV of k-tile j) and are dripped last-deadline-first is not needed;
q (needed at next chunk's start), then k, then V, then Wo.
"""

import contextlib
import itertools
import numpy as np
import ml_dtypes

B, N, D = 2, 2048, 1024
H, DH = 16, 64
NCORES = 8
CPB = 4                  # cores per batch
HPC = H // CPB           # heads per core
F = HPC * DH             # feature slice per core (256)
T = N                    # tokens per core (one batch)
NPAIR = HPC // 2         # head pairs per core
SCALE = DH ** -0.5
P = 128
KC = D // P              # k-chunks over the model dim
NT = T // 512            # 512-token chunks
QC = N // 512            # q chunks per batch
JT = N // P              # 128-token k tiles per batch

BF16 = ml_dtypes.bfloat16

_BUILT = {}


def _build_program(debug=False, loop=0):
    import concourse.bass as bass
    import concourse.tile as tile
    from concourse import mybir
    from concourse.bacc import Bacc

    f32 = mybir.dt.float32
    bf16 = mybir.dt.bfloat16
    EXP = mybir.ActivationFunctionType.Exp

    class BaccOneActTable(Bacc):
        """Force every activation onto the natural_log_exp_and_others table
        set (it contains Exp, Ln, Copy and Identity) so the ACT engine loads
        its function table exactly once instead of thrashing between the
        exp and ln sets (~2.7us per reload)."""

        def insert_act_table_loads(self):
            import bass_rust as _bass_rust
            from concourse.hw_specs import get_activation_tables

            has_activation = any(
                isinstance(i, mybir.InstActivation)
                for blk in self.main_func.blocks
                for i in blk.instructions
            )
            if not has_activation:
                return
            keep = "natural_log_exp_and_others"
            tables = [
                (nm, (fns if nm == keep else set()))
                for nm, fns in get_activation_tables(self.m.arch).items()
            ]
            _bass_rust.insert_act_table_loads(self, tables)

    nc = BaccOneActTable()
    xt = nc.declare_dram_parameter("xt", [D, T], bf16, isOutput=False)
    wq = nc.declare_dram_parameter("wq", [D, F], bf16, isOutput=False)
    wk = nc.declare_dram_parameter("wk", [D, F], bf16, isOutput=False)
    wv = nc.declare_dram_parameter("wv", [D, F], bf16, isOutput=False)
    wo = nc.declare_dram_parameter("wo", [F, D], bf16, isOutput=False)
    out = nc.declare_dram_parameter("out", [T, D], bf16, isOutput=True)

    with tile.TileContext(nc) as tc:
        with contextlib.ExitStack() as ctx:
            persist = ctx.enter_context(tc.tile_pool(name="persist", bufs=1))
            work = ctx.enter_context(tc.tile_pool(name="work", bufs=1))
            dpool = ctx.enter_context(
                tc.tile_pool(name="dscratch", bufs=1, space="DRAM"))

            # ---- persistent SBUF tensors ----
            xt_sb = persist.tile([P, KC, T], bf16)          # x^T, chunked over D
            wq_sb = persist.tile([P, KC, F], bf16)
            wk_sb = persist.tile([P, KC, F], bf16)
            wv_sb = persist.tile([P, KC, F], bf16)
            wo_sb = persist.tile([P, NPAIR, D], bf16)
            qt_sb = persist.tile([P, NPAIR, T], bf16)       # Q^T (scaled)
            kt_sb = persist.tile([P, NPAIR, T], bf16)       # K^T
            # V per 128-tok k-tile: 4 head slots of [V_h (64 cols) | ones]
            va_sb = persist.tile([P, JT, HPC, DH + 1], bf16)
            ot_sb = persist.tile([P, NPAIR, T], bf16)       # normalized O^T
            et_all = persist.tile([P, 5, 1024], bf16)       # E^T rotation bufs
            # causal keep-mask for the diagonal 128x128 block: 1 iff c >= p
            trimask = persist.tile([P, P], bf16)
            ones64 = persist.tile([DH + 1, DH], f32)        # PE bcast lhsT

            # ---- constants ----
            nc.gpsimd.memset(trimask, 1.0)
            nc.gpsimd.affine_select(
                out=trimask, in_=trimask,
                compare_op=mybir.AluOpType.is_ge,
                fill=0.0, base=0, pattern=[[1, P]], channel_multiplier=-1,
            )
            nc.gpsimd.memset(va_sb[:, :, :, DH], 1.0)
            nc.gpsimd.memset(et_all, 0.0)
            nc.gpsimd.memset(ones64, 1.0)

            psum = tc.alloc_tile_pool(name="psum", bufs=1, space="PSUM")

            def body():
                # ---- HAM pre-warm: dense dummy matmuls during the input-DMA
                # window so the PE clock-gate is at 8/8 (2.4 GHz) by the time
                # the first projection matmul issues (saves the ~2us cold
                # penalty; outputs are garbage and never read) ----
                warm = psum.tile([P, P], f32, tag="wop", bufs=1, name="warm")
                for w in range(20):
                    nc.tensor.matmul(warm, trimask, trimask,
                                     start=True, stop=True)

                # ---- load inputs; first projection chunk's needs first ----
                xt_r = xt.rearrange("(a p) t -> p a t", p=P)
                nc.sync.dma_start(out=xt_sb[:, :, 0:512], in_=xt_r[:, :, 0:512])
                nc.sync.dma_start(out=wq_sb, in_=wq.rearrange("(a p) f -> p a f", p=P))
                nc.sync.dma_start(out=wk_sb, in_=wk.rearrange("(a p) f -> p a f", p=P))
                nc.sync.dma_start(out=wv_sb, in_=wv.rearrange("(a p) f -> p a f", p=P))
                for nt in range(1, NT):
                    nc.sync.dma_start(
                        out=xt_sb[:, :, nt * 512:(nt + 1) * 512],
                        in_=xt_r[:, :, nt * 512:(nt + 1) * 512])
                nc.sync.dma_start(out=wo_sb, in_=wo.rearrange("(a p) f -> p a f", p=P))

                # ---- projection / Wo generator streams (for dripping) ----
                def qk_stream(nt, p2, w_sb, dst, ptag, nm):
                    c0 = nt * 512
                    fsl = slice(p2 * P, (p2 + 1) * P)
                    pp = psum.tile([P, 512], f32, tag=ptag, bufs=1,
                                   name=f"pj{nt}{p2}{nm}")
                    for kc in range(KC):
                        nc.tensor.matmul(
                            pp, w_sb[:, kc, fsl], xt_sb[:, kc, c0:c0 + 512],
                            start=(kc == 0), stop=(kc == KC - 1))
                        yield
                    nc.vector.tensor_copy(dst[:, p2, c0:c0 + 512], pp)
                    yield

                def v_stream(tt, ptag):
                    t0 = tt * P
                    pv = psum.tile([P, HPC, DH], f32, tag=ptag, bufs=1,
                                   name=f"pv{tt}")
                    for kc in range(KC):
                        nc.tensor.matmul(
                            pv, xt_sb[:, kc, t0:t0 + P], wv_sb[:, kc, :],
                            start=(kc == 0), stop=(kc == KC - 1))
                        if kc % 2 == 1:
                            yield
                    nc.vector.tensor_copy(va_sb[:, tt, :, 0:DH], pv)
                    yield

                def wo_stream(t0, ptag, pbufs=1):
                    # half-token-tile units so the PSUM tile is one bank
                    for nn in range(2):
                        nsl = slice(nn * 512, (nn + 1) * 512)
                        wop = psum.tile([P, 512], f32, tag=ptag, bufs=pbufs,
                                        name=f"wop{t0}_{nn}")
                        nc.tensor.matmul(wop, ot_sb[:, 0, t0:t0 + P],
                                         wo_sb[:, 0, nsl], start=True,
                                         stop=False)
                        yield
                        nc.tensor.matmul(wop, ot_sb[:, 1, t0:t0 + P],
                                         wo_sb[:, 1, nsl], start=False,
                                         stop=True)
                        yield
                        wos = work.tile([P, 512], bf16, tag="wos", bufs=3,
                                        name=f"wos{t0}_{nn}")
                        nc.vector.tensor_copy(wos, wop)
                        nc.sync.dma_start(out=out[t0:t0 + P, nsl], in_=wos)
                        yield

                def stream_for(qc):
                    """Work dripped during attn(qc), allocated so the PE-idle
                    slack of the late (ACT-bound, large-jmax) q-chunks is
                    filled while respecting deadlines: q(c) by attn(c) start,
                    k(c)/V(c) by attn(c)'s k-tile 4c, wo(c) any time after
                    attn(c)'s normalization. Earliest-deadline work first
                    within each stream."""
                    gens = []
                    tags = itertools.cycle(["wop", "wop2"])

                    def q_gens(c):
                        return [qk_stream(c, p2, wq_sb, qt_sb, next(tags), "q")
                                for p2 in range(NPAIR)]

                    def k_gens(c):
                        return [qk_stream(c, p2, wk_sb, kt_sb, next(tags), "k")
                                for p2 in range(NPAIR)]

                    def v_gens(c):
                        return [v_stream(c * 4 + j4, next(tags))
                                for j4 in range(4)]

                    def wo_gens(c):
                        return [wo_stream(c * 512 + ti * P, next(tags))
                                for ti in range(4)]

                    if qc == 0:
                        gens = q_gens(1) + k_gens(1) + v_gens(1)
                        steps = 56
                    elif qc == 1:
                        gens = q_gens(2) + k_gens(2) + v_gens(2)
                        steps = 56
                    elif qc == 2:
                        gens = q_gens(3) + k_gens(3) + v_gens(3) + wo_gens(0)
                        steps = 80
                    else:
                        gens = wo_gens(1) + wo_gens(2)
                        steps = 48
                    return itertools.chain(*gens), steps

                # proj(0) upfront, rotating psum tags so groups pipeline
                tags = itertools.cycle(["wop", "wop2", "ov0", "ov1"])
                for p2 in range(NPAIR):
                    for _ in qk_stream(0, p2, wq_sb, qt_sb, next(tags), "q"):
                        pass
                    for _ in qk_stream(0, p2, wk_sb, kt_sb, next(tags), "k"):
                        pass
                for j4 in range(4):
                    for _ in v_stream(j4, next(tags)):
                        pass

                et_idx = 0
                for qc in range(QC):
                    qg = qc * 512
                    jmax = (qc + 1) * 4       # k tiles needed (causal)
                    stream, nsteps = stream_for(qc)
                    ntile = NPAIR * jmax
                    tdone = 0
                    sdone = 0

                    def drip(target):
                        nonlocal sdone
                        while sdone < target:
                            if next(stream, None) is None:
                                sdone = max(sdone, nsteps)
                                break
                            sdone += 1

                    def drain():
                        nonlocal sdone
                        for _ in stream:
                            sdone += 1

                    for pr in range(NPAIR):
                        last = (qc == QC - 1 and pr == NPAIR - 1)
                        ovc = work.tile([DH + 1, 1024], f32, tag="ovc",
                                        bufs=2, name=f"ovc{qc}_{pr}")
                        ovs = []
                        for hh in range(2):
                            ov = psum.tile([P, 512], f32, tag=f"ov{hh}",
                                           bufs=1, name=f"ov{hh}_{qc}{pr}")
                            ovs.append(ov[0:DH + 1, :])  # [O(0:64); den(64)]
                        def emit_ev(j, pfx, et):
                            for hh in range(2):
                                nc.tensor.matmul(
                                    ovs[hh][:, pfx:512],
                                    va_sb[:, j, 2 * pr + hh, :],
                                    et[:, hh * 512 + pfx:(hh + 1) * 512],
                                    start=(j == 0), stop=(j == jmax - 1))

                        # EV emission lags QK by two k-tiles so the PE never
                        # waits on exp(j) -- not even across pair boundaries
                        # where the ACT runs the ~2us reciprocal chain first;
                        # the drip between them fills the remaining slack.
                        pend = []
                        for j in range(jmax):
                            kg = j * P
                            # pfx: fully-masked q-column prefix of this tile
                            pfx = max(0, (j - qc * 4)) * P
                            st = psum.tile([P, 1024], f32, tag="st", bufs=2,
                                           name=f"st{qc}_{pr}_{j}")
                            et = et_all[:, et_idx % 5, :]
                            et_idx += 1
                            for hh in range(2):
                                hsl = slice(hh * DH, (hh + 1) * DH)
                                nc.tensor.matmul(
                                    st[:, hh * 512 + pfx:(hh + 1) * 512],
                                    kt_sb[hsl, pr, kg:kg + P],
                                    qt_sb[hsl, pr, qg + pfx:qg + 512],
                                    start=True, stop=True)
                            if pfx == 0:
                                nc.scalar.activation(et, st, EXP)
                            else:
                                for hh in range(2):
                                    esl = slice(hh * 512 + pfx, (hh + 1) * 512)
                                    nc.scalar.activation(et[:, esl], st[:, esl],
                                                         EXP)
                            if j >= qc * 4:   # diag: causal mask
                                for hh in range(2):
                                    blk = slice(hh * 512 + pfx,
                                                hh * 512 + pfx + P)
                                    nc.vector.tensor_mul(
                                        et[:, blk], et[:, blk], trimask)
                            tdone += 1
                            drip(nsteps * tdone // ntile)
                            pend.append((j, pfx, et))
                            if len(pend) > 2:
                                emit_ev(*pend.pop(0))
                        for ev in pend:
                            emit_ev(*ev)
                        # move [O; den] out of PSUM right away so the ov
                        # slots free for the next pair/q-chunk
                        nc.vector.tensor_copy(ovc[:, 0:512], ovs[0])
                        nc.vector.tensor_copy(ovc[:, 512:1024], ovs[1])
                        # 1/den = exp(-ln(den)) on ACT. ACT op cost scales
                        # with per-partition free dim, so reshape the 1024
                        # dens (one partition) to [8,128] via a DRAM bounce
                        # before Ln/Exp (~0.3us/op instead of ~1us), then
                        # broadcast over the 64 feat partitions with a
                        # 0-stride AP. The final pair keeps the 1-partition
                        # Ln/Exp + PE ones-matmul (shorter critical tail).
                        if last:
                            lse = work.tile([DH + 1, 1024], f32, tag="lse",
                                            bufs=2, name=f"lse{qc}_{pr}")
                            nc.scalar.activation(
                                lse[DH:DH + 1, :], ovc[DH:DH + 1, :],
                                mybir.ActivationFunctionType.Ln)
                            rden = work.tile([DH + 1, 1024], f32, tag="rden",
                                             bufs=2, name=f"rden{qc}_{pr}")
                            nc.scalar.activation(
                                rden[DH:DH + 1, :], lse[DH:DH + 1, :],
                                mybir.ActivationFunctionType.Exp, scale=-1.0)
                            rbc_ps = psum.tile([DH, 1024], f32, tag="st",
                                               bufs=2, name="rbcps")
                            for nn in range(2):
                                nsl = slice(nn * 512, (nn + 1) * 512)
                                nc.tensor.matmul(
                                    rbc_ps[:, nsl], ones64[DH:DH + 1, :],
                                    rden[DH:DH + 1, nsl],
                                    start=True, stop=True)
                            rbc = rbc_ps
                        else:
                            dsc = dpool.tile([1, 1024], f32, tag="dsc", bufs=2,
                                             name=f"dsc{qc}_{pr}")
                            nc.sync.dma_start(out=dsc, in_=ovc[DH:DH + 1, :])
                            den8 = work.tile([8, P], f32, tag="den8", bufs=2,
                                             name=f"den8_{qc}_{pr}")
                            nc.sync.dma_start(out=den8, in_=bass.AP(
                                tensor=dsc.tensor, offset=dsc.offset,
                                ap=[[P, 8], [1, P]]))
                            lse8 = work.tile([8, P], f32, tag="lse8", bufs=2,
                                             name=f"lse8_{qc}_{pr}")
                            nc.scalar.activation(
                                lse8, den8, mybir.ActivationFunctionType.Ln)
                            rden8 = work.tile([8, P], f32, tag="rden8", bufs=2,
                                              name=f"rden8_{qc}_{pr}")
                            nc.scalar.activation(
                                rden8, lse8, mybir.ActivationFunctionType.Exp,
                                scale=-1.0)
                            dsc2 = dpool.tile([8, P], f32, tag="dsc2", bufs=2,
                                              name=f"dsc2_{qc}_{pr}")
                            nc.sync.dma_start(out=dsc2, in_=rden8)
                            rbc = work.tile([DH, 1024], f32, tag="rbc", bufs=2,
                                            name=f"rbc{qc}_{pr}")
                            bc_ap = bass.AP(
                                tensor=dsc2.tensor, offset=dsc2.offset,
                                ap=[[0, DH], [1, 1024]])
                            nc.sync.dma_start(out=rbc, in_=bc_ap)
                        nc.vector.tensor_mul(
                            ot_sb[0:DH, pr, qg:qg + 512], ovc[0:DH, 0:512],
                            rbc[:, 0:512])
                        otb = work.tile([DH, 512], bf16, tag="otb", bufs=2,
                                        name=f"otb{qc}_{pr}")
                        nc.vector.tensor_mul(otb, ovc[0:DH, 512:1024],
                                             rbc[:, 512:1024])
                        nc.sync.dma_start(out=ot_sb[DH:P, pr, qg:qg + 512],
                                          in_=otb)
                    drain()
                # tail: Wo for the last q-chunk on the (now free) st slots
                for ti in range(4):
                    for _ in wo_stream((QC - 1) * 512 + ti * P, "st", 2):
                        pass

            if loop:
                ET = mybir.EngineType
                with tc.For_i(0, loop, 1,
                              hint_engines=(ET.PE, ET.DVE, ET.Activation,
                                            ET.SP, ET.Pool)):
                    body()
            else:
                body()
            psum.release()

    nc.finalize()
    return nc


def _get_program():
    if "nc" not in _BUILT:
        _BUILT["nc"] = _build_program()
    return _BUILT["nc"]


def _prep_inputs(x, Wq, Wkv, Wo):
    maps = []
    xts = [np.ascontiguousarray(x[b].T).astype(BF16) for b in range(B)]
    for c in range(NCORES):
        b, g = divmod(c, CPB)
        r0 = g * F
        maps.append({
            "xt": xts[b],
            "wq": np.ascontiguousarray(Wq[r0:r0 + F, :].T * SCALE).astype(BF16),
            "wk": np.ascontiguousarray(Wkv[r0:r0 + F, :].T).astype(BF16),
            "wv": np.ascontiguousarray(Wkv[D + r0:D + r0 + F, :].T).astype(BF16),
            "wo": np.ascontiguousarray(Wo[:, r0:r0 + F].T).astype(BF16),
        })
    return maps


def kernel(x, Wq, Wkv, Wo):
    from concourse.bass_utils import run_bass_kernel_spmd

    nc = _get_program()
    in_maps = _prep_inputs(np.asarray(x, np.float32), np.asarray(Wq, np.float32),
                           np.asarray(Wkv, np.float32), np.asarray(Wo, np.float32))
    res = run_bass_kernel_spmd(nc, in_maps, list(range(NCORES)))
    acc = np.zeros((B, N, D), np.float32)
    for c in range(NCORES):
        b = c // CPB
        acc[b] += res.results[c]["out"].astype(np.float32)
    return acc
